# revision 15
# baseline (speedup 1.0000x reference)
"""MQA attention (32 query heads, 1 KV head, ALiBi, causal) on 8 trn2 cores.

Sharding: tensor-parallel over query heads (4 heads/core). Wq rows and Wo
columns are sharded; x, Wk, Wv are replicated. Each core computes a partial
[T, E] output (its 4 heads pushed through its Wo column-shard); the host sums
the 8 partials.

Math per core c (heads g = 4c..4c+3):
  qT_h = (Wq_h * D^-0.5) @ x^T                     [64, T]   (per head)
  kT   = Wk @ x^T                                  [64, T]
  v    = (Wv @ x^T)^T                              [T, 64]
  ST_h[j, i] = kT[:,j].q  +  (-s_h * i)            via augmented row (ones in kT_aug,
                                                    -s_h*i row in qT_aug)
  g = exp(ST + s_h*j)      (per-partition ACT bias; per-column factors cancel
                            in softmax normalization; causal mask via
                            affine_select fill 0 on diagonal blocks)
  OT_aug = [v | 1]^T @ g   -> rows 0:64 = unnormalized head out^T,
                              row 64    = softmax denominator
  headout^T = OT / denom   (partition_broadcast of 1/denom)
  partial = headout^T.T @ WoT_shard                [T, E]

All matmuls run as float32r (TF32-like PE mode, 4x faster than fp32).
Per-column fp32r rounding of the -s_h*i row cancels in normalization.
"""

import numpy as np

import concourse.bacc as bacc
import concourse.bass as bass
import concourse.mybir as mybir
import concourse.tile as tile
from concourse.bass_utils import run_bass_kernel_spmd
from concourse.masks import make_identity

T = 2048          # tokens
E = 2048          # embed dim
H = 32            # query heads
D = 64            # head dim
NCORES = 8
HL = H // NCORES  # 4 heads per core
ES = HL * D       # 256 = per-core E shard
TQ = 512          # moving-operand tile (max fp32 N)
NTQ = T // TQ     # 4
NE = E // 128     # 16 contraction chunks
NT128 = T // 128  # 16

F32 = mybir.dt.float32
F32R = mybir.dt.float32r
EXP = mybir.ActivationFunctionType.Exp

_CACHE = {}


def _build_nc():
    nc = bacc.Bacc("TRN2")
    xT = nc.dram_tensor("xT", [E, T], F32R, kind="ExternalInput")
    wqT = nc.dram_tensor("wqT", [E, ES], F32R, kind="ExternalInput")
    wkvT = nc.dram_tensor("wkvT", [E, 2 * D], F32R, kind="ExternalInput")
    woT = nc.dram_tensor("woT", [ES, E], F32R, kind="ExternalInput")
    qrow = nc.dram_tensor("qrow", [HL, T], F32R, kind="ExternalInput")
    ones = nc.dram_tensor("ones", [1, T], F32R, kind="ExternalInput")
    btbl = nc.dram_tensor("btbl", [128, HL * NT128], F32, kind="ExternalInput")
    part = nc.dram_tensor("part", [T, E], F32, kind="ExternalOutput")

    from contextlib import ExitStack
    with tile.TileContext(nc) as tc, ExitStack() as ctx:
        _body(nc, tc, ctx, xT, wqT, wkvT, woT, qrow, ones, btbl, part)
    nc.finalize()
    return nc


def _body(nc, tc, ctx, xT, wqT, wkvT, woT, qrow, ones, btbl, part):
    const = ctx.enter_context(tc.tile_pool(name="const", bufs=1))
    xtp = ctx.enter_context(tc.tile_pool(name="xt", bufs=3))
    stg = ctx.enter_context(tc.tile_pool(name="stg", bufs=2))
    gp = ctx.enter_context(tc.tile_pool(name="g", bufs=4))
    dnp = ctx.enter_context(tc.tile_pool(name="dn", bufs=2))
    bcp = ctx.enter_context(tc.tile_pool(name="bc", bufs=2))
    osp = ctx.enter_context(tc.tile_pool(name="ostage", bufs=3))

    # ---------- resident constants ----------
    wq_res = const.tile([128, NE, ES], F32R)
    nc.sync.dma_start(out=wq_res, in_=wqT[:, :].rearrange("(e p) o -> p e o", p=128))
    wkv_res = const.tile([128, NE, 2 * D], F32R)
    nc.sync.dma_start(out=wkv_res, in_=wkvT[:, :].rearrange("(e p) o -> p e o", p=128))
    wo_res = []
    for p2 in range(2):
        w = const.tile([128, E], F32R, tag=f"wo{p2}")
        nc.sync.dma_start(out=w, in_=woT[p2 * 128:(p2 + 1) * 128, :])
        wo_res.append(w)
    qTa = []
    for h in range(HL):
        qa = const.tile([65, T], F32R, tag=f"qTa{h}")
        nc.sync.dma_start(out=qa[64:65, :], in_=qrow[h:h + 1, :])
        qTa.append(qa)
    kTa = const.tile([65, T], F32R)
    nc.sync.dma_start(out=kTa[64:65, :], in_=ones[0:1, :])
    v_aug = const.tile([128, NT128, D + 1], F32R)
    ones_col = ones[0:1, 0:NT128]
    ones_bcast = bass.AP(
        tensor=ones_col.tensor, offset=ones_col.offset,
        ap=[[0, 128], [1, NT128]])
    nc.sync.dma_start(out=v_aug[:, :, D:D + 1], in_=ones_bcast)
    btbl_t = const.tile([128, HL * NT128], F32)
    nc.sync.dma_start(out=btbl_t, in_=btbl[:, :])
    ident = const.tile([128, 128], F32)
    make_identity(nc, ident)
    otn = []
    for p2 in range(2):
        o = const.tile([128, T], F32R, tag=f"otn{p2}")
        otn.append(o)

    # ---------- projections ----------
    with tc.tile_pool(name="proj_ps", bufs=1, space="PSUM") as pp, \
         tc.tile_pool(name="tr_ps", bufs=2, space="PSUM") as trp:
        _projections(nc, tc, pp, trp, xtp, stg, xT, wq_res, wkv_res,
                     qTa, kTa, v_aug, ident)

    # ---------- attention ----------
    with tc.tile_pool(name="st_ps", bufs=3, space="PSUM") as stp, \
         tc.tile_pool(name="ot_ps", bufs=2, space="PSUM") as otp:
        _attention(nc, tc, stp, otp, gp, dnp, bcp, stg, qTa, kTa, v_aug,
                   btbl_t, otn)

    # ---------- output projection ----------
    with tc.tile_pool(name="out_ps", bufs=3, space="PSUM") as oup:
        _outproj(nc, tc, oup, osp, otn, wo_res, part)


def _projections(nc, tc, pp, trp, xtp, stg, xT, wq_res, wkv_res,
                 qTa, kTa, v_aug, ident):
    for q in range(NTQ):
        cs, ce = q * TQ, (q + 1) * TQ
        ps_q0 = pp.tile([128, TQ], F32, tag="psq0")
        ps_q1 = pp.tile([128, TQ], F32, tag="psq1")
        ps_kv = pp.tile([128, TQ], F32, tag="pskv")
        for e in range(NE):
            xt = xtp.tile([128, TQ], F32R)
            nc.sync.dma_start(out=xt, in_=xT[e * 128:(e + 1) * 128, cs:ce])
            st = (e == 0)
            sp = (e == NE - 1)
            nc.tensor.matmul(ps_q0, wq_res[:, e, 0:128], xt, start=st, stop=sp)
            nc.tensor.matmul(ps_q1, wq_res[:, e, 128:256], xt, start=st, stop=sp)
            nc.tensor.matmul(ps_kv, wkv_res[:, e, :], xt, start=st, stop=sp)
        # qT head drains: even heads direct, odd heads via staging + DMA shift
        nc.scalar.copy(out=qTa[0][0:64, cs:ce], in_=ps_q0[0:64, :])
        st0 = stg.tile([128, TQ], F32R, tag="stq")
        nc.scalar.copy(out=st0[64:128, :], in_=ps_q0[64:128, :])
        nc.sync.dma_start(out=qTa[1][0:64, cs:ce], in_=st0[64:128, :])
        nc.scalar.copy(out=qTa[2][0:64, cs:ce], in_=ps_q1[0:64, :])
        st1 = stg.tile([128, TQ], F32R, tag="stq")
        nc.scalar.copy(out=st1[64:128, :], in_=ps_q1[64:128, :])
        nc.sync.dma_start(out=qTa[3][0:64, cs:ce], in_=st1[64:128, :])
        # k drain
        nc.scalar.copy(out=kTa[0:64, cs:ce], in_=ps_kv[0:64, :])
        # v: transpose [64, 512] -> 4x [128, 64] blocks into v_aug
        stv = stg.tile([128, TQ], F32R, tag="stv")
        nc.scalar.copy(out=stv[64:128, :], in_=ps_kv[64:128, :])
        for m in range(4):
            tr = trp.tile([128, D], F32)
            nc.tensor.transpose(
                tr,
                stv[64:128, m * 128:(m + 1) * 128].bitcast(F32),
                ident[64:128, 64:128],
            )
            nc.scalar.copy(out=v_aug[:, q * 4 + m, 0:D], in_=tr)

def _attention(nc, tc, stp, otp, gp, dnp, bcp, stg, qTa, kTa, v_aug,
               btbl_t, otn):
    for h in range(HL):
        for q in range(NTQ):
            cs = q * TQ
            ntk = 4 * q + 4
            ot = otp.tile([65, TQ], F32, tag="ot")
            for tk in range(ntk):
                m = tk - 4 * q
                lo = max(0, m * 128)
                st = stp.tile([128, TQ], F32, tag="st")
                nc.tensor.matmul(
                    st[:, lo:TQ],
                    kTa[:, tk * 128:(tk + 1) * 128],
                    qTa[h][:, cs + lo:cs + TQ],
                    start=True, stop=True,
                )
                g = gp.tile([128, TQ], F32R, tag="g")
                nc.scalar.activation(
                    out=g[:, lo:TQ], in_=st[:, lo:TQ], func=EXP,
                    bias=btbl_t[:, h * NT128 + tk:h * NT128 + tk + 1],
                    scale=1.0,
                )
                if m >= 0:
                    nc.gpsimd.affine_select(
                        out=g[:, lo:lo + 128], in_=g[:, lo:lo + 128],
                        compare_op=mybir.AluOpType.is_ge,
                        fill=0.0, base=0,
                        pattern=[[1, 128]], channel_multiplier=-1,
                    )
                nc.tensor.matmul(
                    ot[:, lo:TQ], v_aug[:, tk, :], g[:, lo:TQ],
                    start=(tk == 0), stop=(tk == ntk - 1),
                )
            # normalize: rows 0:64 / row 64
            dn = dnp.tile([65, TQ], F32, tag="dn")
            nc.scalar.copy(out=dn[64:65, :], in_=ot[64:65, :])
            dn0 = dnp.tile([1, TQ], F32, tag="dn0")
            nc.sync.dma_start(out=dn0[0:1, :], in_=dn[64:65, :])
            rc = dnp.tile([1, TQ], F32, tag="rc")
            nc.vector.reciprocal(out=rc[0:1, :], in_=dn0[0:1, :])
            bc = bcp.tile([64, TQ], F32)
            nc.gpsimd.partition_broadcast(bc, rc[0:1, :])
            pair, half = h // 2, h % 2
            if half == 0:
                nc.vector.tensor_mul(
                    out=otn[pair][0:64, cs:cs + TQ], in0=ot[0:64, :], in1=bc)
            else:
                so = stg.tile([128, TQ], F32R, tag="sot")
                nc.vector.tensor_mul(out=so[0:64, :], in0=ot[0:64, :], in1=bc)
                nc.sync.dma_start(
                    out=otn[pair][64:128, cs:cs + TQ], in_=so[0:64, :])

def _outproj(nc, tc, oup, osp, otn, wo_res, part):
    for t in range(NT128):
        for o in range(NTQ):
            po = oup.tile([128, TQ], F32)
            nc.tensor.matmul(
                po, otn[0][:, t * 128:(t + 1) * 128],
                wo_res[0][:, o * TQ:(o + 1) * TQ], start=True, stop=False)
            nc.tensor.matmul(
                po, otn[1][:, t * 128:(t + 1) * 128],
                wo_res[1][:, o * TQ:(o + 1) * TQ], start=False, stop=True)
            ob = osp.tile([128, TQ], F32)
            nc.vector.tensor_copy(out=ob, in_=po)
            nc.sync.dma_start(
                out=part[t * 128:(t + 1) * 128, o * TQ:(o + 1) * TQ], in_=ob)


def _prepare_in_maps(x, Wq, Wk, Wv, Wo):
    xTn = np.ascontiguousarray(x[0].T).astype(np.float32)
    wkvTn = np.ascontiguousarray(np.concatenate([Wk, Wv], axis=0).T).astype(np.float32)
    scale = np.float32(D ** -0.5)
    in_maps = []
    for c in range(NCORES):
        sl = slice(c * ES, (c + 1) * ES)
        wqTn = np.ascontiguousarray((Wq[sl, :] * scale).T).astype(np.float32)
        woTn = np.ascontiguousarray(Wo[:, sl].T).astype(np.float32)
        g = np.arange(c * HL, (c + 1) * HL, dtype=np.float64)
        slopes = np.power(2.0, -8.0 * (g + 1.0) / H)  # [HL]
        i = np.arange(T, dtype=np.float64)
        qrow_n = (-slopes[:, None] * i[None, :]).astype(np.float32)  # [HL, T]
        p = np.arange(128, dtype=np.float64)
        k = np.arange(NT128, dtype=np.float64)
        # btbl[p, h*16+k] = s_h * (k*128 + p)
        btbl_n = (slopes[:, None, None] * (k[None, :, None] * 128 + p[None, None, :]))
        btbl_n = np.ascontiguousarray(
            btbl_n.transpose(2, 0, 1).reshape(128, HL * NT128)).astype(np.float32)
        in_maps.append({
            "xT": xTn, "wqT": wqTn, "wkvT": wkvTn, "woT": woTn,
            "qrow": qrow_n, "ones": np.ones((1, T), dtype=np.float32),
            "btbl": btbl_n,
        })
    return in_maps


def kernel(x, Wq, Wk, Wv, Wo, attention_mask, _trace=False, _trace_cores=None):
    x = np.asarray(x, dtype=np.float32)
    Wq = np.asarray(Wq, dtype=np.float32)
    Wk = np.asarray(Wk, dtype=np.float32)
    Wv = np.asarray(Wv, dtype=np.float32)
    Wo = np.asarray(Wo, dtype=np.float32)

    if "nc" not in _CACHE:
        _CACHE["nc"] = _build_nc()
    nc = _CACHE["nc"]

    in_maps = _prepare_in_maps(x, Wq, Wk, Wv, Wo)
    kwargs = {}
    if _trace:
        kwargs = {"trace": True, "trace_cores": _trace_cores or [0]}
    res = run_bass_kernel_spmd(nc, in_maps, core_ids=list(range(NCORES)), **kwargs)
    acc = np.zeros((T, E), dtype=np.float64)
    for r in res.results:
        acc += r["part"].astype(np.float64)
    out = acc.astype(np.float32)[None, :, :]
    if _trace:
        _CACHE["last_result"] = res
    return out


# revision 19
# speedup vs baseline: 1.1170x; 1.1170x over previous
"""MQA attention (32 query heads, 1 KV head, ALiBi, causal) on 8 trn2 cores.

Sharding: tensor-parallel over query heads (4 heads/core). Wq rows and Wo
columns are sharded; x, Wk, Wv are replicated. Each core computes a partial
[T, E] output (its 4 heads pushed through its Wo column-shard); the host sums
the 8 partials.

Math per core c (heads g = 4c..4c+3):
  qT_h = (Wq_h * D^-0.5) @ x^T                     [64, T]   (per head)
  kT   = Wk @ x^T                                  [64, T]
  v    = (Wv @ x^T)^T                              [T, 64]
  ST_h[j, i] = kT[:,j].q  +  (-s_h * i)            via augmented row (ones in kT_aug,
                                                    -s_h*i row in qT_aug)
  g = exp(ST + s_h*j)      (per-partition ACT bias; per-column factors cancel
                            in softmax normalization; causal mask via
                            affine_select fill 0 on diagonal blocks)
  OT_aug = [v | 1]^T @ g   -> rows 0:64 = unnormalized head out^T,
                              row 64    = softmax denominator
  headout^T = OT / denom   (partition_broadcast of 1/denom)
  partial = headout^T.T @ WoT_shard                [T, E]

All matmuls run as float32r (TF32-like PE mode, 4x faster than fp32).
Per-column fp32r rounding of the -s_h*i row cancels in normalization.
"""

import numpy as np

import concourse.bacc as bacc
import concourse.bass as bass
import concourse.mybir as mybir
import concourse.tile as tile
from concourse.masks import make_identity
from concourse.bass_utils import run_bass_kernel_spmd

T = 2048          # tokens
E = 2048          # embed dim
H = 32            # query heads
D = 64            # head dim
NCORES = 8
HL = H // NCORES  # 4 heads per core
ES = HL * D       # 256 = per-core E shard
TQ = 512          # moving-operand tile (max fp32 N)
NTQ = T // TQ     # 4
NE = E // 128     # 16 contraction chunks
NT128 = T // 128  # 16

F32 = mybir.dt.float32
F32R = mybir.dt.float32r
EXP = mybir.ActivationFunctionType.Exp

_CACHE = {}


def _build_nc():
    nc = bacc.Bacc("TRN2")
    xT = nc.dram_tensor("xT", [E, T], F32R, kind="ExternalInput")
    wqT = nc.dram_tensor("wqT", [E, ES], F32R, kind="ExternalInput")
    wkvT = nc.dram_tensor("wkvT", [E, 2 * D], F32R, kind="ExternalInput")
    woT = nc.dram_tensor("woT", [ES, E], F32R, kind="ExternalInput")
    qrow = nc.dram_tensor("qrow", [HL, T], F32R, kind="ExternalInput")
    ones = nc.dram_tensor("ones", [1, T], F32R, kind="ExternalInput")
    btbl = nc.dram_tensor("btbl", [128, HL * NT128], F32, kind="ExternalInput")
    part = nc.dram_tensor("part", [T, E], F32, kind="ExternalOutput")

    from contextlib import ExitStack
    with tile.TileContext(nc) as tc, ExitStack() as ctx:
        _body(nc, tc, ctx, xT, wqT, wkvT, woT, qrow, ones, btbl, part)
    nc.finalize()
    return nc


def _body(nc, tc, ctx, xT, wqT, wkvT, woT, qrow, ones, btbl, part):
    const = ctx.enter_context(tc.tile_pool(name="const", bufs=1))
    xtp = ctx.enter_context(tc.tile_pool(name="xt", bufs=20))
    stg = ctx.enter_context(tc.tile_pool(name="stg", bufs=2))
    gp = ctx.enter_context(tc.tile_pool(name="g", bufs=4))
    dnp = ctx.enter_context(tc.tile_pool(name="dn", bufs=2))
    bcp = ctx.enter_context(tc.tile_pool(name="bc", bufs=2))
    osp = ctx.enter_context(tc.tile_pool(name="ostage", bufs=3))

    # ---------- resident constants ----------
    wq_res = const.tile([128, NE, ES], F32R)
    nc.sync.dma_start(out=wq_res, in_=wqT[:, :].rearrange("(e p) o -> p e o", p=128))
    wkv_res = const.tile([128, NE, 2 * D], F32R)
    nc.sync.dma_start(out=wkv_res, in_=wkvT[:, :].rearrange("(e p) o -> p e o", p=128))
    wo_res = []
    for p2 in range(2):
        w = const.tile([128, E], F32R, tag=f"wo{p2}")
        nc.sync.dma_start(out=w, in_=woT[p2 * 128:(p2 + 1) * 128, :])
        wo_res.append(w)
    qTa = []
    for h in range(HL):
        qa = const.tile([65, T], F32R, tag=f"qTa{h}")
        nc.sync.dma_start(out=qa[64:65, :], in_=qrow[h:h + 1, :])
        qTa.append(qa)
    kTa = const.tile([65, T], F32R)
    nc.sync.dma_start(out=kTa[64:65, :], in_=ones[0:1, :])
    v_aug = const.tile([128, NT128, D + 1], F32R)
    ones_col = ones[0:1, 0:NT128]
    ones_bcast = bass.AP(
        tensor=ones_col.tensor, offset=ones_col.offset,
        ap=[[0, 128], [1, NT128]])
    nc.sync.dma_start(out=v_aug[:, :, D:D + 1], in_=ones_bcast)
    btbl_t = const.tile([128, HL * NT128], F32)
    nc.sync.dma_start(out=btbl_t, in_=btbl[:, :])
    ident = const.tile([128, 128], F32)
    make_identity(nc, ident)
    otn = []
    for p2 in range(2):
        o = const.tile([128, T], F32R, tag=f"otn{p2}")
        otn.append(o)

    # ---------- 8 PSUM banks total: pacc 2 + st 2 + ot 2 + out 2 ----------
    pp = ctx.enter_context(tc.tile_pool(name="pacc", bufs=2, space="PSUM"))
    stp = ctx.enter_context(tc.tile_pool(name="st_ps", bufs=2, space="PSUM"))
    otp = ctx.enter_context(tc.tile_pool(name="ot_ps", bufs=2, space="PSUM"))
    oup = ctx.enter_context(tc.tile_pool(name="out_ps", bufs=2, space="PSUM"))

    _projections(nc, tc, pp, stp, xtp, stg, xT, wq_res, wkv_res, qTa, kTa, v_aug, ident)
    _attention(nc, tc, stp, otp, gp, dnp, bcp, stg, qTa, kTa, v_aug,
               btbl_t, otn)
    _outproj(nc, tc, oup, osp, otn, wo_res, part)


def _projections(nc, tc, pp, stp, xtp, stg, xT, wq_res, wkv_res, qTa, kTa, v_aug, ident):
    for q in range(NTQ):
        cs, ce = q * TQ, (q + 1) * TQ
        xts = []
        for e in range(NE):
            xt = xtp.tile([128, TQ], F32R, tag="xt")
            nc.sync.dma_start(out=xt, in_=xT[e * 128:(e + 1) * 128, cs:ce])
            xts.append(xt)
        # group 0: heads 0/1
        acc = pp.tile([128, TQ], F32, tag="acc")
        for e in range(NE):
            nc.tensor.matmul(acc, wq_res[:, e, 0:128], xts[e],
                             start=(e == 0), stop=(e == NE - 1))
        nc.vector.tensor_copy(out=qTa[0][0:64, cs:ce], in_=acc[0:64, :])
        st0 = stg.tile([128, TQ], F32R, tag="stq")
        nc.vector.tensor_copy(out=st0[64:128, :], in_=acc[64:128, :])
        nc.sync.dma_start(out=qTa[1][0:64, cs:ce], in_=st0[64:128, :])
        # group 1: heads 2/3
        acc = pp.tile([128, TQ], F32, tag="acc")
        for e in range(NE):
            nc.tensor.matmul(acc, wq_res[:, e, 128:256], xts[e],
                             start=(e == 0), stop=(e == NE - 1))
        nc.vector.tensor_copy(out=qTa[2][0:64, cs:ce], in_=acc[0:64, :])
        st1 = stg.tile([128, TQ], F32R, tag="stq")
        nc.vector.tensor_copy(out=st1[64:128, :], in_=acc[64:128, :])
        nc.sync.dma_start(out=qTa[3][0:64, cs:ce], in_=st1[64:128, :])
        # group 2: k (rows 0:64) and v (rows 64:128)
        acc = pp.tile([128, TQ], F32, tag="acc")
        for e in range(NE):
            nc.tensor.matmul(acc, wkv_res[:, e, :], xts[e],
                             start=(e == 0), stop=(e == NE - 1))
        nc.vector.tensor_copy(out=kTa[0:64, cs:ce], in_=acc[0:64, :])
        stv = stg.tile([128, TQ], F32R, tag="stv")
        nc.vector.tensor_copy(out=stv[64:128, :], in_=acc[64:128, :])
        # v transpose via PE: 4x [64, 128] -> [128, 64]
        for mm in range(4):
            tr = stp.tile([128, TQ], F32, tag="st")
            nc.tensor.transpose(
                tr[:, 0:D],
                stv[64:128, mm * 128:(mm + 1) * 128].bitcast(F32),
                ident[64:128, 64:128])
            nc.vector.tensor_copy(out=v_aug[:, q * 4 + mm, 0:D], in_=tr[:, 0:D])

def _attention(nc, tc, stp, otp, gp, dnp, bcp, stg, qTa, kTa, v_aug,
               btbl_t, otn):
    for h in range(HL):
        for q in range(NTQ):
            cs = q * TQ
            ntk = 4 * q + 4
            ot = otp.tile([65, TQ], F32, tag="ot")
            for tk in range(ntk):
                m = tk - 4 * q
                # pad narrow diag tiles to >=256 cols (fp32r 1 cyc/row zone)
                lo = min(max(0, m * 128), TQ - 256)
                st = stp.tile([128, TQ], F32, tag="st")
                nc.tensor.matmul(
                    st[:, lo:TQ],
                    kTa[:, tk * 128:(tk + 1) * 128],
                    qTa[h][:, cs + lo:cs + TQ],
                    start=True, stop=True,
                )
                g = gp.tile([128, TQ], F32R, tag="g")
                nc.scalar.activation(
                    out=g[:, lo:TQ], in_=st[:, lo:TQ], func=EXP,
                    bias=btbl_t[:, h * NT128 + tk:h * NT128 + tk + 1],
                    scale=1.0,
                )
                if m >= 0:
                    mw = m * 128 + 128 - lo
                    nc.gpsimd.affine_select(
                        out=g[:, lo:lo + mw], in_=g[:, lo:lo + mw],
                        compare_op=mybir.AluOpType.is_ge,
                        fill=0.0, base=lo - m * 128,
                        pattern=[[1, mw]], channel_multiplier=-1,
                    )
                nc.tensor.matmul(
                    ot[:, lo:TQ], v_aug[:, tk, :], g[:, lo:TQ],
                    start=(tk == 0), stop=(tk == ntk - 1),
                )
            # normalize: rows 0:64 / row 64
            dn = dnp.tile([65, TQ], F32, tag="dn")
            nc.scalar.copy(out=dn[64:65, :], in_=ot[64:65, :])
            dn0 = dnp.tile([1, TQ], F32, tag="dn0")
            nc.sync.dma_start(out=dn0[0:1, :], in_=dn[64:65, :])
            rc = dnp.tile([1, TQ], F32, tag="rc")
            nc.vector.reciprocal(out=rc[0:1, :], in_=dn0[0:1, :])
            bc = bcp.tile([64, TQ], F32)
            nc.gpsimd.partition_broadcast(bc, rc[0:1, :])
            pair, half = h // 2, h % 2
            if half == 0:
                nc.vector.tensor_mul(
                    out=otn[pair][0:64, cs:cs + TQ], in0=ot[0:64, :], in1=bc)
            else:
                so = stg.tile([128, TQ], F32R, tag="sot")
                nc.vector.tensor_mul(out=so[0:64, :], in0=ot[0:64, :], in1=bc)
                nc.sync.dma_start(
                    out=otn[pair][64:128, cs:cs + TQ], in_=so[0:64, :])

def _outproj(nc, tc, oup, osp, otn, wo_res, part):
    for t in range(NT128):
        for o in range(NTQ):
            po = oup.tile([128, TQ], F32)
            nc.tensor.matmul(
                po, otn[0][:, t * 128:(t + 1) * 128],
                wo_res[0][:, o * TQ:(o + 1) * TQ], start=True, stop=False)
            nc.tensor.matmul(
                po, otn[1][:, t * 128:(t + 1) * 128],
                wo_res[1][:, o * TQ:(o + 1) * TQ], start=False, stop=True)
            ob = osp.tile([128, TQ], F32)
            nc.vector.tensor_copy(out=ob, in_=po)
            nc.sync.dma_start(
                out=part[t * 128:(t + 1) * 128, o * TQ:(o + 1) * TQ], in_=ob)


def _prepare_in_maps(x, Wq, Wk, Wv, Wo):
    xTn = np.ascontiguousarray(x[0].T).astype(np.float32)
    wkvTn = np.ascontiguousarray(np.concatenate([Wk, Wv], axis=0).T).astype(np.float32)
    scale = np.float32(D ** -0.5)
    in_maps = []
    for c in range(NCORES):
        sl = slice(c * ES, (c + 1) * ES)
        wqTn = np.ascontiguousarray((Wq[sl, :] * scale).T).astype(np.float32)
        woTn = np.ascontiguousarray(Wo[:, sl].T).astype(np.float32)
        g = np.arange(c * HL, (c + 1) * HL, dtype=np.float64)
        slopes = np.power(2.0, -8.0 * (g + 1.0) / H)  # [HL]
        i = np.arange(T, dtype=np.float64)
        qrow_n = (-slopes[:, None] * i[None, :]).astype(np.float32)  # [HL, T]
        p = np.arange(128, dtype=np.float64)
        k = np.arange(NT128, dtype=np.float64)
        # btbl[p, h*16+k] = s_h * (k*128 + p)
        btbl_n = (slopes[:, None, None] * (k[None, :, None] * 128 + p[None, None, :]))
        btbl_n = np.ascontiguousarray(
            btbl_n.transpose(2, 0, 1).reshape(128, HL * NT128)).astype(np.float32)
        in_maps.append({
            "xT": xTn, "wqT": wqTn, "wkvT": wkvTn, "woT": woTn,
            "qrow": qrow_n, "ones": np.ones((1, T), dtype=np.float32),
            "btbl": btbl_n,
        })
    return in_maps


def kernel(x, Wq, Wk, Wv, Wo, attention_mask, _trace=False, _trace_cores=None):
    x = np.asarray(x, dtype=np.float32)
    Wq = np.asarray(Wq, dtype=np.float32)
    Wk = np.asarray(Wk, dtype=np.float32)
    Wv = np.asarray(Wv, dtype=np.float32)
    Wo = np.asarray(Wo, dtype=np.float32)

    if "nc" not in _CACHE:
        _CACHE["nc"] = _build_nc()
    nc = _CACHE["nc"]

    in_maps = _prepare_in_maps(x, Wq, Wk, Wv, Wo)
    kwargs = {}
    if _trace:
        kwargs = {"trace": True, "trace_cores": _trace_cores or [0]}
    res = run_bass_kernel_spmd(nc, in_maps, core_ids=list(range(NCORES)), **kwargs)
    acc = np.zeros((T, E), dtype=np.float64)
    for r in res.results:
        acc += r["part"].astype(np.float64)
    out = acc.astype(np.float32)[None, :, :]
    if _trace:
        _CACHE["last_result"] = res
    return out


# revision 57
# speedup vs baseline: 1.5131x; 1.3546x over previous
"""MQA attention (32 query heads, 1 KV head, ALiBi, causal) on 8 trn2 cores.

Sharding: tensor-parallel over query heads (4 heads/core). Wq rows and Wo
columns are sharded; x, Wk, Wv are replicated. Each core computes a partial
[T, E] output (its 4 heads pushed through its Wo column-shard); the host sums
the 8 partials.

Math per core c (heads g = 4c..4c+3):
  qT_h = (Wq_h * D^-0.5) @ x^T                     [64, T]   (per head)
  kT   = Wk @ x^T                                  [64, T]
  v    = (Wv @ x^T)^T                              [T, 64]
  ST_h[j, i] = kT[:,j].q  +  (-s_h * i)            via augmented row (ones in kT_aug,
                                                    -s_h*i row in qT_aug)
  g = exp(ST + s_h*j)      (per-partition ACT bias; per-column factors cancel
                            in softmax normalization; causal mask via
                            affine_select fill 0 on diagonal blocks)
  OT_aug = [v | 1]^T @ g   -> rows 0:64 = unnormalized head out^T,
                              row 64    = softmax denominator
  headout^T = OT / denom   (partition_broadcast of 1/denom)
  partial = headout^T.T @ WoT_shard                [T, E]

All matmuls run as float32r (TF32-like PE mode, 4x faster than fp32).
Per-column fp32r rounding of the -s_h*i row cancels in normalization.
"""

import numpy as np

import concourse.bacc as bacc
import concourse.bass as bass
import concourse.mybir as mybir
import concourse.tile as tile
from concourse.masks import make_identity
from concourse.bass_utils import run_bass_kernel_spmd

T = 2048          # tokens
E = 2048          # embed dim
H = 32            # query heads
D = 64            # head dim
NCORES = 8
HL = H // NCORES  # 4 heads per core
ES = HL * D       # 256 = per-core E shard
TQ = 512          # moving-operand tile (max fp32 N)
NTQ = T // TQ     # 4
NE = E // 128     # 16 contraction chunks
NT128 = T // 128  # 16

F32 = mybir.dt.float32
F32R = mybir.dt.float32r
EXP = mybir.ActivationFunctionType.Exp

_CACHE = {}


def _build_nc(debug=False):
    nc = bacc.Bacc("TRN2")
    xT = nc.dram_tensor("xT", [E, T], F32R, kind="ExternalInput")
    wqT = nc.dram_tensor("wqT", [E, ES], F32R, kind="ExternalInput")
    wkvT = nc.dram_tensor("wkvT", [E, 2 * D], F32R, kind="ExternalInput")
    woT = nc.dram_tensor("woT", [ES, E], F32R, kind="ExternalInput")
    qrow = nc.dram_tensor("qrow", [HL, T], F32R, kind="ExternalInput")
    ones = nc.dram_tensor("ones", [1, T], F32R, kind="ExternalInput")
    btbl = nc.dram_tensor("btbl", [128, HL * NT128], F32, kind="ExternalInput")
    part = nc.dram_tensor("part", [T, E], F32, kind="ExternalOutput")
    dbg = None
    if debug:
        dbg = {
            "k": nc.dram_tensor("dbg_k", [65, T], F32, kind="ExternalOutput"),
            "v": nc.dram_tensor("dbg_v", [128, NT128, D + 1], F32, kind="ExternalOutput"),
            "otn": nc.dram_tensor("dbg_otn", [2, 128, T], F32, kind="ExternalOutput"),
        }

    from contextlib import ExitStack
    with tile.TileContext(nc) as tc, ExitStack() as ctx:
        _body(nc, tc, ctx, xT, wqT, wkvT, woT, qrow, ones, btbl, part, dbg=dbg)
    nc.finalize()
    return nc


def _body(nc, tc, ctx, xT, wqT, wkvT, woT, qrow, ones, btbl, part, dbg=None):
    const = ctx.enter_context(tc.tile_pool(name="const", bufs=1))
    xtp = ctx.enter_context(tc.tile_pool(name="xt", bufs=20))
    stg = ctx.enter_context(tc.tile_pool(name="stg", bufs=3))
    gp = ctx.enter_context(tc.tile_pool(name="g", bufs=6))
    dnp = ctx.enter_context(tc.tile_pool(name="dn", bufs=4))
    bcp = ctx.enter_context(tc.tile_pool(name="bc", bufs=4))
    osp = ctx.enter_context(tc.tile_pool(name="ostage", bufs=4))

    # ---------- resident constants (weight DMAs emitted inside phase 1) ----
    wq_res = const.tile([128, NE, ES], F32R)
    wkv_res = const.tile([128, NE, 2 * D], F32R)
    wo_res = []
    for p2 in range(2):
        w = const.tile([128, E], F32R, tag=f"wo{p2}")
        wo_res.append(w)
    qTa = []
    for h in range(HL):
        qa = const.tile([65, T], F32R, tag=f"qTa{h}")
        nc.sync.dma_start(out=qa[64:65, :], in_=qrow[h:h + 1, :])
        qTa.append(qa)
    kTa = const.tile([65, T], F32R)
    nc.sync.dma_start(out=kTa[64:65, :], in_=ones[0:1, :])
    v_aug = const.tile([128, NT128, D + 1], F32R)
    ones_col = ones[0:1, 0:NT128]
    ones_bcast = bass.AP(
        tensor=ones_col.tensor, offset=ones_col.offset,
        ap=[[0, 128], [1, NT128]])
    nc.sync.dma_start(out=v_aug[:, :, D:D + 1], in_=ones_bcast)
    btbl_t = const.tile([128, HL * NT128], F32)
    nc.sync.dma_start(out=btbl_t, in_=btbl[:, :])
    ident = const.tile([128, 128], F32)
    make_identity(nc, ident)
    otn = []
    for p2 in range(2):
        o = const.tile([128, T], F32R, tag=f"otn{p2}")
        otn.append(o)

    # ---------- 8 PSUM banks total: pacc 2 + st 2 + ot 2 + out 2 ----------
    pp = ctx.enter_context(tc.tile_pool(name="pacc", bufs=2, space="PSUM"))
    stp = ctx.enter_context(tc.tile_pool(name="st_ps", bufs=3, space="PSUM"))
    otp = ctx.enter_context(tc.tile_pool(name="ot_ps", bufs=2, space="PSUM"))
    oup = ctx.enter_context(tc.tile_pool(name="out_ps", bufs=1, space="PSUM"))

    for q in range(NTQ):
        _projections_q(nc, q, pp, stp, xtp, stg, xT, wqT, wkvT, woT,
                       wq_res, wkv_res, wo_res, qTa, kTa, v_aug, ident)
        for h in range(HL):
            _attention_hq(nc, h, q, stp, otp, gp, dnp, bcp, stg,
                          qTa, kTa, v_aug, btbl_t, otn)
        for t in range(4 * q, 4 * q + 4):
            _outproj_t(nc, t, oup, stp, osp, otn, wo_res, part)
    if dbg is not None:
        nc.sync.dma_start(out=dbg["k"][:, :], in_=kTa[:, :].bitcast(F32))
        nc.sync.dma_start(out=dbg["v"][:, :, :], in_=v_aug[:, :, :].bitcast(F32))
        for p2 in range(2):
            nc.sync.dma_start(out=dbg["otn"][p2], in_=otn[p2][:, :].bitcast(F32))


def _projections_q(nc, q, pp, stp, xtp, stg, xT, wqT, wkvT, woT,
                   wq_res, wkv_res, wo_res, qTa, kTa, v_aug, ident):
    cs, ce = q * TQ, (q + 1) * TQ
    xts = []
    for e in range(NE):
        if q == 0:
            nc.sync.dma_start(out=wq_res[:, e, :],
                              in_=wqT[e * 128:(e + 1) * 128, :])
            nc.sync.dma_start(out=wkv_res[:, e, :],
                              in_=wkvT[e * 128:(e + 1) * 128, :])
        xt = xtp.tile([128, TQ], F32R, tag="xt")
        eng = nc.gpsimd if q == 0 else nc.sync
        eng.dma_start(out=xt, in_=xT[e * 128:(e + 1) * 128, cs:ce])
        xts.append(xt)
    # group 0: heads 0/1
    acc = pp.tile([128, TQ], F32, tag="acc")
    for e in range(NE):
        nc.tensor.matmul(acc, wq_res[:, e, 0:128], xts[e],
                         start=(e == 0), stop=(e == NE - 1))
    nc.vector.tensor_copy(out=qTa[0][0:64, cs:ce], in_=acc[0:64, :])
    st0 = stg.tile([128, TQ], F32R, tag="stq")
    nc.vector.tensor_copy(out=st0[64:128, :], in_=acc[64:128, :])
    nc.sync.dma_start(out=qTa[1][0:64, cs:ce], in_=st0[64:128, :])
    # group 1: heads 2/3
    acc = pp.tile([128, TQ], F32, tag="acc")
    for e in range(NE):
        nc.tensor.matmul(acc, wq_res[:, e, 128:256], xts[e],
                         start=(e == 0), stop=(e == NE - 1))
    nc.vector.tensor_copy(out=qTa[2][0:64, cs:ce], in_=acc[0:64, :])
    st1 = stg.tile([128, TQ], F32R, tag="stq")
    nc.vector.tensor_copy(out=st1[64:128, :], in_=acc[64:128, :])
    nc.sync.dma_start(out=qTa[3][0:64, cs:ce], in_=st1[64:128, :])
    # group 2: k (rows 0:64) and v (rows 64:128)
    acc = pp.tile([128, TQ], F32, tag="acc")
    for e in range(NE):
        nc.tensor.matmul(acc, wkv_res[:, e, :], xts[e],
                         start=(e == 0), stop=(e == NE - 1))
    nc.vector.tensor_copy(out=kTa[0:64, cs:ce], in_=acc[0:64, :])
    stv = stg.tile([128, TQ], F32R, tag="stv")
    nc.vector.tensor_copy(out=stv[64:128, :], in_=acc[64:128, :])
    # v transpose via PE: 4x [64, 128] -> [128, 64]
    for mm in range(4):
        tr = stp.tile([128, TQ], F32, tag="st")
        nc.tensor.transpose(
            tr[:, 0:D],
            stv[64:128, mm * 128:(mm + 1) * 128].bitcast(F32),
            ident[64:128, 64:128])
        nc.vector.tensor_copy(out=v_aug[:, q * 4 + mm, 0:D], in_=tr[:, 0:D])
    if q == 0:
        for p2 in range(2):
            nc.sync.dma_start(out=wo_res[p2],
                              in_=woT[p2 * 128:(p2 + 1) * 128, :])

def _attention_hq(nc, h, q, stp, otp, gp, dnp, bcp, stg,
                  qTa, kTa, v_aug, btbl_t, otn):
    cs = q * TQ
    ntk = 4 * q + 4
    ot = otp.tile([65, TQ], F32, tag="ot")
    for tk in range(ntk):
        m = tk - 4 * q
        # pad narrow diag tiles to >=256 cols (fp32r 1 cyc/row zone)
        lo = min(max(0, m * 128), TQ - 256)
        st = stp.tile([128, TQ], F32, tag="st")
        nc.tensor.matmul(
            st[:, lo:TQ],
            kTa[:, tk * 128:(tk + 1) * 128],
            qTa[h][:, cs + lo:cs + TQ],
            start=True, stop=True,
        )
        g = gp.tile([128, TQ], F32R, tag="g")
        nc.scalar.activation(
            out=g[:, lo:TQ], in_=st[:, lo:TQ], func=EXP,
            bias=btbl_t[:, h * NT128 + tk:h * NT128 + tk + 1],
            scale=1.0,
        )
        if m >= 0:
            mw = m * 128 + 128 - lo
            nc.gpsimd.affine_select(
                out=g[:, lo:lo + mw], in_=g[:, lo:lo + mw],
                compare_op=mybir.AluOpType.is_ge,
                fill=0.0, base=lo - m * 128,
                pattern=[[1, mw]], channel_multiplier=-1,
            )
        nc.tensor.matmul(
            ot[:, lo:TQ], v_aug[:, tk, :], g[:, lo:TQ],
            start=(tk == 0), stop=(tk == ntk - 1),
        )
    # normalize: rows 0:64 / row 64
    dn = dnp.tile([65, TQ], F32, tag="dn")
    nc.vector.tensor_copy(out=dn[64:65, :], in_=ot[64:65, :])
    dn0 = dnp.tile([1, TQ], F32, tag="dn0")
    nc.sync.dma_start(out=dn0[0:1, :], in_=dn[64:65, :])
    rc = dnp.tile([1, TQ], F32, tag="rc")
    nc.vector.reciprocal(out=rc[0:1, :], in_=dn0[0:1, :])
    bc = bcp.tile([64, TQ], F32)
    nc.gpsimd.partition_broadcast(bc, rc[0:1, :])
    pair, half = h // 2, h % 2
    if half == 0:
        nc.vector.tensor_mul(
            out=otn[pair][0:64, cs:cs + TQ], in0=ot[0:64, :], in1=bc)
    else:
        so = stg.tile([128, TQ], F32R, tag="sot")
        nc.vector.tensor_mul(out=so[0:64, :], in0=ot[0:64, :], in1=bc)
        nc.sync.dma_start(
            out=otn[pair][64:128, cs:cs + TQ], in_=so[0:64, :])

def _outproj_t(nc, t, oup, stp, osp, otn, wo_res, part):
    for o in range(NTQ):
        if t >= 12 and o % 2 == 1:
            po = stp.tile([128, TQ], F32, tag="st")
        else:
            po = oup.tile([128, TQ], F32, tag="po")
        nc.tensor.matmul(
            po, otn[0][:, t * 128:(t + 1) * 128],
            wo_res[0][:, o * TQ:(o + 1) * TQ], start=True, stop=False)
        nc.tensor.matmul(
            po, otn[1][:, t * 128:(t + 1) * 128],
            wo_res[1][:, o * TQ:(o + 1) * TQ], start=False, stop=True)
        ob = osp.tile([128, TQ], F32)
        if t >= 12:
            nc.vector.tensor_copy(out=ob[:, 0:256], in_=po[:, 0:256])
            nc.scalar.copy(out=ob[:, 256:TQ], in_=po[:, 256:TQ])
        else:
            nc.vector.tensor_copy(out=ob, in_=po)
        nc.sync.dma_start(
            out=part[t * 128:(t + 1) * 128, o * TQ:(o + 1) * TQ], in_=ob)
